# revision 1
# baseline (speedup 1.0000x reference)
"""Trainium2 Bass kernel for EvalHead (NMS detection decode).

Computes, for x [B=16, C=15, H=512, W=512] fp32:
  scores = x[:,0]; peak = (scores > 0.5) & (scores == maxpool3x3(scores))
  out[b,h,w,:] = [score, cx-hx, cy-hy, cx+hx, cy+hy, lm0x+px, lm0y+py, ...] * peak
  where cx = px + x[:,1], cy = py + x[:,2], hx = 0.5*x[:,3], hy = 0.5*x[:,4],
        px = 4*w+2, py = 4*h+2.
Output: [16, 512, 512, 15] fp32.

Sharding: pure data parallel over batch — 2 images per core across 8 cores.
Per-core layout: partition = image row; 4 tiles of [128 rows, 512 cols] per
image. Vertical pool via +-1-row shifted HBM loads (edge rows clamped, since
max(a,a,b)=max(a,b) matches SAME padding); horizontal pool via shifted
free-dim slices of an edge-duplicated padded tile. Output assembled in SBUF
channel-interleaved [128, 512*15] so the store is one contiguous DMA.
"""

import numpy as np

B = 16
N_CORES = 8
B_LOCAL = B // N_CORES  # 2 images per core
C = 15
H = 512
W = 512
PT = 128                 # partition tile height (rows)
NT = H // PT             # 4 row-tiles per image
STRIDE = 4
OFF_Y = 2.0
OFF_X = 2.0
THRESHOLD = 0.5
NEG = -1e30

_CACHE = {}


def _build_nc():
    from contextlib import ExitStack

    import bass_rust
    import concourse.tile as tile
    from concourse import bacc, mybir
    from concourse.alu_op_type import AluOpType

    f32 = mybir.dt.float32
    Act = bass_rust.ActivationFunctionType

    nc = bacc.Bacc(None, target_bir_lowering=False)

    x = nc.dram_tensor("x", [B_LOCAL, C, H, W], f32, kind="ExternalInput")
    pxd = nc.dram_tensor("pxd", [PT, W], f32, kind="ExternalInput")
    pyd = nc.dram_tensor("pyd", [NT, PT], f32, kind="ExternalInput")
    out = nc.dram_tensor("out", [B_LOCAL, H, W, C], f32, kind="ExternalOutput")

    with tile.TileContext(nc) as tc, ExitStack() as ctx:
        const = ctx.enter_context(tc.tile_pool(name="const", bufs=1))
        inp = ctx.enter_context(tc.tile_pool(name="inp", bufs=2))
        sp = ctx.enter_context(tc.tile_pool(name="sp", bufs=2))
        mid = ctx.enter_context(tc.tile_pool(name="mid", bufs=1))
        tmp = ctx.enter_context(tc.tile_pool(name="tmp", bufs=1))
        outp = ctx.enter_context(tc.tile_pool(name="outp", bufs=2))

        pxt = const.tile([PT, W], f32)
        nc.sync.dma_start(pxt[:], pxd[:])
        pyt = const.tile([PT, NT], f32)
        nc.sync.dma_start(pyt[:], pyd.rearrange("t p -> p t"))
        # px broadcast views: [p][j][w] with j (landmark idx) as a 0-step dim
        pxb = pxt[:].broadcast_to([PT, W, 5]).rearrange("p w j -> p j w")

        for b in range(B_LOCAL):
            for t in range(NT):
                r0 = PT * t

                in15 = inp.tile([PT, C * W], f32)
                v15 = in15.rearrange("p (c w) -> p c w", c=C)
                nc.sync.dma_start(v15[:, :, :], x[b, :, r0:r0 + PT, :].rearrange("c p w -> p c w"))

                sup = sp.tile([PT, W], f32)
                if t > 0:
                    nc.sync.dma_start(sup[:], x[b, 0, r0 - 1:r0 + PT - 1, :])
                else:
                    nc.sync.dma_start(sup[0:1, :], x[b, 0, 0:1, :])
                    nc.sync.dma_start(sup[1:PT, :], x[b, 0, 0:PT - 1, :])
                sdn = sp.tile([PT, W], f32)
                if t < NT - 1:
                    nc.sync.dma_start(sdn[:], x[b, 0, r0 + 1:r0 + PT + 1, :])
                else:
                    nc.sync.dma_start(sdn[0:PT - 1, :], x[b, 0, r0 + 1:H, :])
                    nc.sync.dma_start(sdn[PT - 1:PT, :], x[b, 0, H - 1:H, :])

                smid = v15[:, 0, :]

                # ---- 3x3 max pool -> peak mask m ----
                v1 = mid.tile([PT, W], f32)
                nc.vector.tensor_tensor(v1[:], sup[:], sdn[:], op=AluOpType.max)
                vp = mid.tile([PT, W + 2], f32)
                nc.vector.tensor_tensor(vp[:, 1:W + 1], v1[:], smid, op=AluOpType.max)
                # duplicate-edge pad: max(v0,v0,v1) == max(v0,v1) == SAME pooling
                nc.vector.tensor_copy(vp[:, 0:1], vp[:, 1:2])
                nc.vector.tensor_copy(vp[:, W + 1:W + 2], vp[:, W:W + 1])
                t2 = mid.tile([PT, W], f32)
                nc.vector.tensor_tensor(t2[:], vp[:, 0:W], vp[:, 1:W + 1], op=AluOpType.max)
                pooled = mid.tile([PT, W], f32)
                nc.vector.tensor_tensor(pooled[:], t2[:], vp[:, 2:W + 2], op=AluOpType.max)
                eq = mid.tile([PT, W], f32)
                nc.vector.tensor_tensor(eq[:], smid, pooled[:], op=AluOpType.is_equal)
                m = mid.tile([PT, W], f32)
                nc.vector.scalar_tensor_tensor(
                    m[:], smid, THRESHOLD, eq[:], AluOpType.is_gt, AluOpType.mult)
                mb = m[:].broadcast_to([PT, W, 5]).rearrange("p w j -> p j w")

                # ---- decode ----
                pycol = pyt[:, t:t + 1]
                cxp = mid.tile([PT, W], f32)
                nc.gpsimd.tensor_tensor(cxp[:], v15[:, 1, :], pxt[:], op=AluOpType.add)
                cyp = mid.tile([PT, W], f32)
                nc.scalar.activation(cyp[:], v15[:, 2, :], Act.Identity, bias=pycol, scale=1.0)
                tx1 = mid.tile([PT, W], f32)
                nc.vector.scalar_tensor_tensor(
                    tx1[:], v15[:, 3, :], -0.5, cxp[:], AluOpType.mult, AluOpType.add)
                tx2 = mid.tile([PT, W], f32)
                nc.vector.scalar_tensor_tensor(
                    tx2[:], v15[:, 3, :], 0.5, cxp[:], AluOpType.mult, AluOpType.add)
                ty1 = mid.tile([PT, W], f32)
                nc.vector.scalar_tensor_tensor(
                    ty1[:], v15[:, 4, :], -0.5, cyp[:], AluOpType.mult, AluOpType.add)
                ty2 = mid.tile([PT, W], f32)
                nc.vector.scalar_tensor_tensor(
                    ty2[:], v15[:, 4, :], 0.5, cyp[:], AluOpType.mult, AluOpType.add)

                # landmarks: channels 5..14 = 5 (x, y) pairs
                lmp = v15[:, 5:C, :].rearrange("p (j k) w -> p j k w", k=2)
                tlx = tmp.tile([PT, 5 * W], f32)
                tlxv = tlx.rearrange("p (j w) -> p j w", j=5)
                nc.gpsimd.tensor_tensor(tlxv[:, :, :], lmp[:, :, 0, :], pxb, op=AluOpType.add)
                tly = tmp.tile([PT, 5 * W], f32)
                tlyv = tly.rearrange("p (j w) -> p j w", j=5)
                nc.scalar.activation(tlyv[:, :, :], lmp[:, :, 1, :], Act.Identity, bias=pycol, scale=1.0)

                # ---- masked interleaved output ----
                ot = outp.tile([PT, W * C], f32)
                ot4 = ot.rearrange("p (w c) -> p w c", c=C)
                nc.vector.tensor_tensor(ot4[:, :, 0], smid, m[:], op=AluOpType.mult)
                nc.vector.tensor_tensor(ot4[:, :, 1], tx1[:], m[:], op=AluOpType.mult)
                nc.vector.tensor_tensor(ot4[:, :, 2], ty1[:], m[:], op=AluOpType.mult)
                nc.vector.tensor_tensor(ot4[:, :, 3], tx2[:], m[:], op=AluOpType.mult)
                nc.vector.tensor_tensor(ot4[:, :, 4], ty2[:], m[:], op=AluOpType.mult)
                olm = ot4[:, :, 5:C].rearrange("p w (j k) -> p w j k", k=2)
                nc.vector.tensor_tensor(
                    olm[:, :, :, 0].rearrange("p w j -> p j w"), tlxv[:, :, :], mb, op=AluOpType.mult)
                nc.vector.tensor_tensor(
                    olm[:, :, :, 1].rearrange("p w j -> p j w"), tlyv[:, :, :], mb, op=AluOpType.mult)

                nc.sync.dma_start(out[b, r0:r0 + PT, :, :], ot4[:, :, :])

    nc.compile()
    return nc


def _aux_inputs():
    pxd = (np.arange(W, dtype=np.float32) * STRIDE + OFF_X)[None, :].repeat(PT, 0)
    pyd = (np.arange(H, dtype=np.float32) * STRIDE + OFF_Y).reshape(NT, PT)
    return np.ascontiguousarray(pxd), np.ascontiguousarray(pyd)


def kernel(x: np.ndarray) -> np.ndarray:
    from concourse.bass_utils import run_bass_kernel_spmd

    if "nc" not in _CACHE:
        _CACHE["nc"] = _build_nc()
    nc = _CACHE["nc"]

    x = np.ascontiguousarray(np.asarray(x, dtype=np.float32))
    assert x.shape == (B, C, H, W), x.shape
    pxd, pyd = _aux_inputs()
    in_maps = [
        {"x": np.ascontiguousarray(x[i * B_LOCAL:(i + 1) * B_LOCAL]), "pxd": pxd, "pyd": pyd}
        for i in range(N_CORES)
    ]
    res = run_bass_kernel_spmd(nc, in_maps, list(range(N_CORES)))
    return np.concatenate([res.results[i]["out"] for i in range(N_CORES)], axis=0)


# revision 2
# speedup vs baseline: 48784.4551x; 48784.4551x over previous
"""Trainium2 Bass kernel for EvalHead (NMS detection decode).

Computes, for x [B=16, C=15, H=512, W=512] fp32:
  scores = x[:,0]; peak = (scores > 0.5) & (scores == maxpool3x3(scores))
  out[b,h,w,:] = [score, cx-hx, cy-hy, cx+hx, cy+hy, lm0x+px, lm0y+py, ...] * peak
  where cx = px + x[:,1], cy = py + x[:,2], hx = 0.5*x[:,3], hy = 0.5*x[:,4],
        px = 4*w+2, py = 4*h+2.
Output: [16, 512, 512, 15] fp32.

Sharding: pure data parallel over batch — 2 images per core across 8 cores.
Per-core layout: partition = image row; 4 tiles of [128 rows, 512 cols] per
image. Vertical pool via +-1-row shifted HBM loads (edge rows clamped, since
max(a,a,b)=max(a,b) matches SAME padding); horizontal pool via shifted
free-dim slices of an edge-duplicated padded tile. Output assembled in SBUF
channel-interleaved [128, 512*15] so the store is one contiguous DMA.
"""

import numpy as np

B = 16
N_CORES = 8
B_LOCAL = B // N_CORES  # 2 images per core
C = 15
H = 512
W = 512
PT = 128                 # partition tile height (rows)
NT = H // PT             # 4 row-tiles per image
STRIDE = 4
OFF_Y = 2.0
OFF_X = 2.0
THRESHOLD = 0.5
NEG = -1e30

_CACHE = {}


def _build_nc(loop_k: int = 1):
    """Build the per-core Bass module. loop_k > 1 wraps the whole body in a
    hardware For loop (used only for timing measurements)."""
    from contextlib import ExitStack, nullcontext

    import bass_rust
    import concourse.tile as tile
    from concourse import bacc, mybir
    from concourse.alu_op_type import AluOpType

    f32 = mybir.dt.float32
    Act = bass_rust.ActivationFunctionType

    nc = bacc.Bacc(None, target_bir_lowering=False)

    x = nc.dram_tensor("x", [B_LOCAL, C, H, W], f32, kind="ExternalInput")
    pxd = nc.dram_tensor("pxd", [PT, W], f32, kind="ExternalInput")
    pyd = nc.dram_tensor("pyd", [NT, PT], f32, kind="ExternalInput")
    out = nc.dram_tensor("out", [B_LOCAL, H, W, C], f32, kind="ExternalOutput")

    with tile.TileContext(nc) as tc, ExitStack() as ctx:
        loop = tc.For_i(0, loop_k, 1) if loop_k > 1 else nullcontext()
        ctx.enter_context(loop)
        const = ctx.enter_context(tc.tile_pool(name="const", bufs=1))
        inp = ctx.enter_context(tc.tile_pool(name="inp", bufs=2))
        sp = ctx.enter_context(tc.tile_pool(name="sp", bufs=2))
        mid = ctx.enter_context(tc.tile_pool(name="mid", bufs=1))
        tmp = ctx.enter_context(tc.tile_pool(name="tmp", bufs=1))
        outp = ctx.enter_context(tc.tile_pool(name="outp", bufs=2))

        pxt = const.tile([PT, W], f32)
        nc.sync.dma_start(pxt[:], pxd[:])
        pyt = const.tile([PT, NT], f32)
        nc.sync.dma_start(pyt[:], pyd.rearrange("t p -> p t"))
        # px broadcast views: [p][j][w] with j (landmark idx) as a 0-step dim
        pxb = pxt[:].broadcast_to([PT, W, 5]).rearrange("p w j -> p j w")

        for b in range(B_LOCAL):
            for t in range(NT):
                r0 = PT * t

                in15 = inp.tile([PT, C * W], f32)
                v15 = in15.rearrange("p (c w) -> p c w", c=C)
                nc.sync.dma_start(v15[:, :, :], x[b, :, r0:r0 + PT, :].rearrange("c p w -> p c w"))

                sup = sp.tile([PT, W], f32)
                if t > 0:
                    nc.sync.dma_start(sup[:], x[b, 0, r0 - 1:r0 + PT - 1, :])
                else:
                    nc.sync.dma_start(sup[0:1, :], x[b, 0, 0:1, :])
                    nc.sync.dma_start(sup[1:PT, :], x[b, 0, 0:PT - 1, :])
                sdn = sp.tile([PT, W], f32)
                if t < NT - 1:
                    nc.sync.dma_start(sdn[:], x[b, 0, r0 + 1:r0 + PT + 1, :])
                else:
                    nc.sync.dma_start(sdn[0:PT - 1, :], x[b, 0, r0 + 1:H, :])
                    nc.sync.dma_start(sdn[PT - 1:PT, :], x[b, 0, H - 1:H, :])

                smid = v15[:, 0, :]

                # ---- 3x3 max pool -> peak mask m ----
                v1 = mid.tile([PT, W], f32)
                nc.vector.tensor_tensor(v1[:], sup[:], sdn[:], op=AluOpType.max)
                vp = mid.tile([PT, W + 2], f32)
                nc.vector.tensor_tensor(vp[:, 1:W + 1], v1[:], smid, op=AluOpType.max)
                # duplicate-edge pad: max(v0,v0,v1) == max(v0,v1) == SAME pooling
                nc.vector.tensor_copy(vp[:, 0:1], vp[:, 1:2])
                nc.vector.tensor_copy(vp[:, W + 1:W + 2], vp[:, W:W + 1])
                t2 = mid.tile([PT, W], f32)
                nc.vector.tensor_tensor(t2[:], vp[:, 0:W], vp[:, 1:W + 1], op=AluOpType.max)
                pooled = mid.tile([PT, W], f32)
                nc.vector.tensor_tensor(pooled[:], t2[:], vp[:, 2:W + 2], op=AluOpType.max)
                eq = mid.tile([PT, W], f32)
                nc.vector.tensor_tensor(eq[:], smid, pooled[:], op=AluOpType.is_equal)
                m = mid.tile([PT, W], f32)
                nc.vector.scalar_tensor_tensor(
                    m[:], smid, THRESHOLD, eq[:], AluOpType.is_gt, AluOpType.mult)
                mb = m[:].broadcast_to([PT, W, 5]).rearrange("p w j -> p j w")

                # ---- decode ----
                pycol = pyt[:, t:t + 1]
                cxp = mid.tile([PT, W], f32)
                nc.gpsimd.tensor_tensor(cxp[:], v15[:, 1, :], pxt[:], op=AluOpType.add)
                cyp = mid.tile([PT, W], f32)
                nc.scalar.activation(cyp[:], v15[:, 2, :], Act.Identity, bias=pycol, scale=1.0)
                tx1 = mid.tile([PT, W], f32)
                nc.vector.scalar_tensor_tensor(
                    tx1[:], v15[:, 3, :], -0.5, cxp[:], AluOpType.mult, AluOpType.add)
                tx2 = mid.tile([PT, W], f32)
                nc.vector.scalar_tensor_tensor(
                    tx2[:], v15[:, 3, :], 0.5, cxp[:], AluOpType.mult, AluOpType.add)
                ty1 = mid.tile([PT, W], f32)
                nc.vector.scalar_tensor_tensor(
                    ty1[:], v15[:, 4, :], -0.5, cyp[:], AluOpType.mult, AluOpType.add)
                ty2 = mid.tile([PT, W], f32)
                nc.vector.scalar_tensor_tensor(
                    ty2[:], v15[:, 4, :], 0.5, cyp[:], AluOpType.mult, AluOpType.add)

                # landmarks: channels 5..14 = 5 (x, y) pairs
                lmp = v15[:, 5:C, :].rearrange("p (j k) w -> p j k w", k=2)
                tlx = tmp.tile([PT, 5 * W], f32)
                tlxv = tlx.rearrange("p (j w) -> p j w", j=5)
                nc.gpsimd.tensor_tensor(tlxv[:, :, :], lmp[:, :, 0, :], pxb, op=AluOpType.add)
                tly = tmp.tile([PT, 5 * W], f32)
                tlyv = tly.rearrange("p (j w) -> p j w", j=5)
                nc.scalar.activation(tlyv[:, :, :], lmp[:, :, 1, :], Act.Identity, bias=pycol, scale=1.0)

                # ---- masked interleaved output ----
                ot = outp.tile([PT, W * C], f32)
                ot4 = ot.rearrange("p (w c) -> p w c", c=C)
                nc.vector.tensor_tensor(ot4[:, :, 0], smid, m[:], op=AluOpType.mult)
                nc.vector.tensor_tensor(ot4[:, :, 1], tx1[:], m[:], op=AluOpType.mult)
                nc.vector.tensor_tensor(ot4[:, :, 2], ty1[:], m[:], op=AluOpType.mult)
                nc.vector.tensor_tensor(ot4[:, :, 3], tx2[:], m[:], op=AluOpType.mult)
                nc.vector.tensor_tensor(ot4[:, :, 4], ty2[:], m[:], op=AluOpType.mult)
                olm = ot4[:, :, 5:C].rearrange("p w (j k) -> p w j k", k=2)
                nc.vector.tensor_tensor(
                    olm[:, :, :, 0].rearrange("p w j -> p j w"), tlxv[:, :, :], mb, op=AluOpType.mult)
                nc.vector.tensor_tensor(
                    olm[:, :, :, 1].rearrange("p w j -> p j w"), tlyv[:, :, :], mb, op=AluOpType.mult)

                nc.sync.dma_start(out[b, r0:r0 + PT, :, :], ot4[:, :, :])

    nc.compile()
    return nc


def _aux_inputs():
    pxd = (np.arange(W, dtype=np.float32) * STRIDE + OFF_X)[None, :].repeat(PT, 0)
    pyd = (np.arange(H, dtype=np.float32) * STRIDE + OFF_Y).reshape(NT, PT)
    return np.ascontiguousarray(pxd), np.ascontiguousarray(pyd)


def kernel(x: np.ndarray) -> np.ndarray:
    from concourse.bass_utils import run_bass_kernel_spmd

    if "nc" not in _CACHE:
        _CACHE["nc"] = _build_nc()
    nc = _CACHE["nc"]

    x = np.ascontiguousarray(np.asarray(x, dtype=np.float32))
    assert x.shape == (B, C, H, W), x.shape
    pxd, pyd = _aux_inputs()
    in_maps = [
        {"x": np.ascontiguousarray(x[i * B_LOCAL:(i + 1) * B_LOCAL]), "pxd": pxd, "pyd": pyd}
        for i in range(N_CORES)
    ]
    res = run_bass_kernel_spmd(nc, in_maps, list(range(N_CORES)))
    return np.concatenate([res.results[i]["out"] for i in range(N_CORES)], axis=0)
